# revision 21
# baseline (speedup 1.0000x reference)
"""Trainium2 Bass kernel for nn_Blur (upfirdn2d: up=2, pad=(2,1,2,1), 4-tap
separable filter [1,3,3,1] x [1,3,3,1] / 64).

Input  x [16, 128, 128, 128] f32  ->  Output [16, 128, 256, 256] f32.

Math (polyphase decomposition of the zero-insertion upsample + conv), per
axis with zero boundary:
  even outputs:  y[2i]   = (1*x[i-1] + 3*x[i]) / 8
  odd  outputs:  y[2i+1] = (3*x[i]   + 1*x[i+1]) / 8

The kernel is HBM-bound, so all device I/O is bf16 (gate is rel_err < 2e-2;
the full bf16 rounding chain measures ~3e-3):
  in  8 MB/core + out 32 MB/core = 40 MB/core @ ~358 GB/s  ->  ~112 us floor
(f32 baseline moved 80 MB/core -> 269 us).

Pipeline per group of GROUP=8 channel-images:
  pass 1 (vertical) on TensorE: V = A.T @ X, A [128, 256] bf16 banded
     polyphase matrix carrying the full 1/64 scale; V in PSUM f32 laid out
     [p, (ph, i, j)] (ph = output row parity, p = row pair index).
  pass 2 (horizontal) split for engine balance (ACT ~ DVE ~ 109 us each,
     both under the DMA floor):
   - ACT: u = 3V -> SBUF bf16 (one op), plus part of Vb = copy(V) into
     zero-padded 130-wide blocks (pads persist across groups: K rotating
     tiles memset once at startup).
   - DVE: rest of the Vb copy (1x, PSUM operand), then ONE fused add in
     bf16 2x mode (all operands 16-bit SBUF stride-1, 4B-aligned) over a
     plane dim that is broadcast (stride 0) on u and selects the +-1
     shift (stride 2 elements) on Vb:
        E[j] = u[j] + Vb[j-1]   (even output cols)
        O[j] = u[j] + Vb[j+1]   (odd  output cols)
     Zero pads make the j=0/127 boundaries fall out with no edge ops.
  store: two DMAs per group (E plane, O plane), 4 KB/partition bf16 each.

Host side: input is pre-permuted+cast to bf16 [32, h, i, w] per core so
loads are 2 KB/partition contiguous; output planes are gathered/interleaved
back to f32 NCHW with a threaded strided-cast pass.

Sharding: pure data parallel, 2 examples (256 channel-images) per core.
"""

import numpy as np

H = 128
W = 128
N_CORES = 8
NIMG_PER_CORE = 2 * 128  # 256 channel-images per core
GROUP = 8
NGROUPS = NIMG_PER_CORE // GROUP  # 32
NBLK = 2 * GROUP       # (ph, i) blocks per group
BLKW = W + 2           # padded Vb block width
LEAD = 12              # input loads issued this many groups ahead
VB_BUFS = 3            # persistent padded-Vb tiles (pads memset once)
ACT_VB_BLOCKS = 10     # Vb blocks copied by ACT; rest (NBLK-this) by DVE


def _filter_matrix() -> np.ndarray:
    """A[h, m] bf16-exact: m in 0..127 -> even output row 2m; m in 128..255
    -> odd row 2(m-128)+1. Carries the full 1/64 separable scale."""
    A = np.zeros((H, 2 * H), np.float32)
    for i in range(H):
        if i - 1 >= 0:
            A[i - 1, i] = 1.0 / 64
        A[i, i] = 3.0 / 64
        A[i, H + i] = 3.0 / 64
        if i + 1 < H:
            A[i + 1, H + i] = 1.0 / 64
    return A


def build_kernel_body(tc, x, filt, out, ngroups):
    """x [ngroups, 128, GROUP*W] bf16 (h-major, pre-permuted on host),
    filt [128, 256] bf16, out [ngroups, 128, 2*NBLK*W] bf16."""
    from contextlib import ExitStack

    import concourse.mybir as mybir
    from concourse.ap import AP

    bf16 = mybir.dt.bfloat16
    f32 = mybir.dt.float32
    nc = tc.nc
    GW = GROUP * W  # 1024

    with ExitStack() as ctx:
        const_pool = ctx.enter_context(tc.tile_pool(name="const", bufs=1))
        xin_pool = ctx.enter_context(tc.tile_pool(name="xin", bufs=LEAD + 2))
        v_pool = ctx.enter_context(tc.tile_pool(name="v", bufs=2, space="PSUM"))
        u_pool = ctx.enter_context(tc.tile_pool(name="u", bufs=4))
        vb_pool = ctx.enter_context(tc.tile_pool(name="vb", bufs=VB_BUFS))
        eo_pool = ctx.enter_context(tc.tile_pool(name="eo", bufs=4))

        A = const_pool.tile([H, 2 * H], bf16)
        nc.sync.dma_start(A[:], filt)

        # persistent padded-Vb tiles; zero the pad columns once
        vb_tiles = [
            vb_pool.tile([H, NBLK * BLKW], bf16, name=f"vb{i}")
            for i in range(VB_BUFS)
        ]
        for vb in vb_tiles:
            pads = vb[:].rearrange("p (b c) -> p b c", c=BLKW)[:, :, 0 : BLKW : BLKW - 1]
            nc.vector.memset(pads, 0.0)

        xg_tiles = {}

        def issue_load(gl):
            if gl >= ngroups:
                return
            xg = xin_pool.tile([H, GW], bf16)
            xg_tiles[gl] = xg
            # loads ride the ACT HWDGE ring (trigger never waits: the xin
            # buffer was freed LEAD+2 groups ago), so prefetch cannot get
            # stuck behind a store blocked on compute on the SP ring
            nc.scalar.dma_start(xg[:], x[gl])

        for gl in range(LEAD):
            issue_load(gl)

        for g in range(ngroups):
            issue_load(g + LEAD)
            xg = xg_tiles.pop(g)

            # pass 1 (vertical): V[p, (ph, i, j)] f32 in PSUM
            v = v_pool.tile([H, 2 * GW], f32)
            for ph in range(2):
                for half in range(2):
                    nc.tensor.matmul(
                        v[:, ph * GW + half * 512 : ph * GW + (half + 1) * 512],
                        A[:, ph * H : (ph + 1) * H],
                        xg[:, half * 512 : (half + 1) * 512],
                        start=True,
                        stop=True,
                    )
            v4 = v[:].rearrange("p (b j) -> p b j", b=NBLK)

            # u = 3V -> bf16 SBUF (ACT)
            u = u_pool.tile([H, 2 * GW], bf16)
            nc.scalar.mul(u[:], v[:], 3.0)

            # Vb = V -> bf16 into padded blocks, split ACT / DVE
            vb = vb_tiles[g % VB_BUFS]
            vb4 = vb[:].rearrange("p (b c) -> p b c", c=BLKW)
            ba = ACT_VB_BLOCKS
            if ba > 0:
                nc.scalar.copy(vb4[:, 0:ba, 1 : W + 1], v4[:, 0:ba, :])
            if ba < NBLK:
                nc.vector.tensor_copy(vb4[:, ba:NBLK, 1 : W + 1], v4[:, ba:NBLK, :])

            # pass 2 adds (DVE, bf16 2x): E = u + Vb[j-1], O = u + Vb[j+1];
            # store each plane as soon as its add completes (4 KB/partition)
            eo = eo_pool.tile([H, 2 * 2 * GW], bf16)
            eo4 = eo[:].rearrange("p (pl b j) -> p pl b j", pl=2, b=NBLK)
            u4 = u[:].rearrange("p (b j) -> p b j", b=NBLK)
            # single DVE op for both planes: pl dim is broadcast (stride 0)
            # on u and selects the +-1 shift (stride 2) on Vb
            u_ap = u4[:, :, :]
            ub = AP(u_ap.tensor, u_ap.offset, [list(u_ap.ap[0]), [0, 2]] + [list(d) for d in u_ap.ap[1:]])
            vbs = vb4[:, :, 0:W]
            vbb = AP(vbs.tensor, vbs.offset, [list(vbs.ap[0]), [2, 2]] + [list(d) for d in vbs.ap[1:]])
            nc.vector.tensor_add(eo4[:, 0:2], ub, vbb)
            # two stores per group (4 KB/partition each, matching the DMA
            # packet ceiling; a single 8 KB store measured ~3 us slower)
            out2 = out[g].rearrange("p (pl c) -> p pl c", pl=2)
            nc.sync.dma_start(out2[:, 0], eo[:, 0 : 2 * GW])
            nc.sync.dma_start(out2[:, 1], eo[:, 2 * GW : 4 * GW])


def build_bass(ngroups=NGROUPS, enable_asserts=False):
    import concourse.bacc as bacc
    import concourse.mybir as mybir
    import concourse.tile as tile

    bf16 = mybir.dt.bfloat16
    nc = bacc.Bacc(
        "TRN2",
        target_bir_lowering=False,
        debug=False,
        enable_asserts=enable_asserts,
        num_devices=N_CORES,
    )
    x = nc.dram_tensor("x", [ngroups, H, GROUP * W], bf16, kind="ExternalInput").ap()
    filt = nc.dram_tensor("filt", [H, 2 * H], bf16, kind="ExternalInput").ap()
    out = nc.dram_tensor(
        "out", [ngroups, H, 2 * NBLK * W], bf16, kind="ExternalOutput"
    ).ap()
    with tile.TileContext(nc) as tc:
        build_kernel_body(tc, x, filt, out, ngroups)
    nc.compile()
    return nc


_NC_CACHE = {}


def _prep_input_core(x_core):
    """[256, 128, 128] f32 -> [32, h, i, w] bf16 contiguous."""
    import ml_dtypes

    v = x_core.reshape(NGROUPS, GROUP, H, W).transpose(0, 2, 1, 3)
    return np.ascontiguousarray(v.astype(ml_dtypes.bfloat16))


def _unpack_output_core(raw, dst):
    """raw [32, 128, 2*NBLK*W] bf16 -> dst [256, 256, 256] f32."""
    a = raw.reshape(NGROUPS, H, 2, 2, GROUP, W)  # g p pl ph i j
    # dst[g*GROUP+i, 2p+ph, 2j+pl]
    np.copyto(
        dst.reshape(NGROUPS, GROUP, H, 2, W, 2),
        a.transpose(0, 4, 1, 3, 5, 2),
        casting="unsafe",
    )


def kernel(x: np.ndarray, _trace=False, _trace_cores=None) -> np.ndarray:
    from concurrent.futures import ThreadPoolExecutor

    import ml_dtypes

    from concourse.bass_utils import run_bass_kernel_spmd

    assert x.shape == (16, 128, H, W), x.shape
    xf = np.ascontiguousarray(x, dtype=np.float32).reshape(
        N_CORES, NIMG_PER_CORE, H, W
    )
    A = _filter_matrix().astype(ml_dtypes.bfloat16)

    with ThreadPoolExecutor(N_CORES) as ex:
        xcores = list(ex.map(_prep_input_core, [xf[k] for k in range(N_CORES)]))
    in_maps = [{"x": xcores[k], "filt": A} for k in range(N_CORES)]

    key = NGROUPS
    if key not in _NC_CACHE:
        _NC_CACHE[key] = build_bass()
    nc = _NC_CACHE[key]

    res = run_bass_kernel_spmd(
        nc,
        in_maps,
        core_ids=list(range(N_CORES)),
        trace=_trace,
        trace_cores=_trace_cores,
    )
    out = np.empty((N_CORES * NIMG_PER_CORE, 2 * H, 2 * W), np.float32)
    with ThreadPoolExecutor(N_CORES) as ex:
        list(
            ex.map(
                lambda k: _unpack_output_core(
                    res.results[k]["out"],
                    out[k * NIMG_PER_CORE : (k + 1) * NIMG_PER_CORE],
                ),
                range(N_CORES),
            )
        )
    if _trace:
        kernel._last_result = res
    return out.reshape(16, 128, 2 * H, 2 * W)


# revision 23
# speedup vs baseline: 1.0120x; 1.0120x over previous
"""Trainium2 Bass kernel for nn_Blur (upfirdn2d: up=2, pad=(2,1,2,1), 4-tap
separable filter [1,3,3,1] x [1,3,3,1] / 64).

Input  x [16, 128, 128, 128] f32  ->  Output [16, 128, 256, 256] f32.

Math (polyphase decomposition of the zero-insertion upsample + conv), per
axis with zero boundary:
  even outputs:  y[2i]   = (1*x[i-1] + 3*x[i]) / 8
  odd  outputs:  y[2i+1] = (3*x[i]   + 1*x[i+1]) / 8

The kernel is HBM-bound, so all device I/O is bf16 (gate is rel_err < 2e-2;
the full bf16 rounding chain measures ~3e-3):
  in  8 MB/core + out 32 MB/core = 40 MB/core @ ~358 GB/s  ->  ~112 us floor
(f32 baseline moved 80 MB/core -> 269 us).

Pipeline per group of GROUP=8 channel-images:
  pass 1 (vertical) on TensorE: V = A.T @ X, A [128, 256] bf16 banded
     polyphase matrix carrying the full 1/64 scale; V in PSUM f32 laid out
     [p, (ph, i, j)] (ph = output row parity, p = row pair index).
  pass 2 (horizontal) split for engine balance (ACT ~ DVE ~ 109 us each,
     both under the DMA floor):
   - ACT: u = 3V -> SBUF bf16 (one op), plus part of Vb = copy(V) into
     zero-padded 130-wide blocks (pads persist across groups: K rotating
     tiles memset once at startup).
   - DVE: rest of the Vb copy (1x, PSUM operand), then ONE fused add in
     bf16 2x mode (all operands 16-bit SBUF stride-1, 4B-aligned) over a
     plane dim that is broadcast (stride 0) on u and selects the +-1
     shift (stride 2 elements) on Vb:
        E[j] = u[j] + Vb[j-1]   (even output cols)
        O[j] = u[j] + Vb[j+1]   (odd  output cols)
     Zero pads make the j=0/127 boundaries fall out with no edge ops.
  store: two DMAs per group (E plane, O plane), 4 KB/partition bf16 each.

Host side: input is pre-permuted+cast to bf16 [32, h, i, w] per core so
loads are 2 KB/partition contiguous; output planes are gathered/interleaved
back to f32 NCHW with a threaded strided-cast pass.

Sharding: pure data parallel, 2 examples (256 channel-images) per core.
"""

import numpy as np

H = 128
W = 128
N_CORES = 8
NIMG_PER_CORE = 2 * 128  # 256 channel-images per core
GROUP = 8
NGROUPS = NIMG_PER_CORE // GROUP  # 32
NBLK = 2 * GROUP       # (ph, i) blocks per group
BLKW = W + 2           # padded Vb block width
BURST = 17             # input loads issued up front; rest rationed 1-per-2-groups
VB_BUFS = 3            # persistent padded-Vb tiles (pads memset once)
ACT_VB_BLOCKS = 10     # Vb blocks copied by ACT; rest (NBLK-this) by DVE


def _filter_matrix() -> np.ndarray:
    """A[h, m] bf16-exact: m in 0..127 -> even output row 2m; m in 128..255
    -> odd row 2(m-128)+1. Carries the full 1/64 separable scale."""
    A = np.zeros((H, 2 * H), np.float32)
    for i in range(H):
        if i - 1 >= 0:
            A[i - 1, i] = 1.0 / 64
        A[i, i] = 3.0 / 64
        A[i, H + i] = 3.0 / 64
        if i + 1 < H:
            A[i + 1, H + i] = 1.0 / 64
    return A


def build_kernel_body(tc, x, filt, out, ngroups):
    """x [ngroups, 128, GROUP*W] bf16 (h-major, pre-permuted on host),
    filt [128, 256] bf16, out [ngroups, 128, 2*NBLK*W] bf16."""
    from contextlib import ExitStack

    import concourse.mybir as mybir
    from concourse.ap import AP

    bf16 = mybir.dt.bfloat16
    f32 = mybir.dt.float32
    nc = tc.nc
    GW = GROUP * W  # 1024

    with ExitStack() as ctx:
        const_pool = ctx.enter_context(tc.tile_pool(name="const", bufs=1))
        xin_pool = ctx.enter_context(tc.tile_pool(name="xin", bufs=BURST + 1))
        v_pool = ctx.enter_context(tc.tile_pool(name="v", bufs=2, space="PSUM"))
        u_pool = ctx.enter_context(tc.tile_pool(name="u", bufs=4))
        vb_pool = ctx.enter_context(tc.tile_pool(name="vb", bufs=VB_BUFS))
        eo_pool = ctx.enter_context(tc.tile_pool(name="eo", bufs=4))

        A = const_pool.tile([H, 2 * H], bf16)
        nc.sync.dma_start(A[:], filt)

        # persistent padded-Vb tiles; zero the pad columns once
        vb_tiles = [
            vb_pool.tile([H, NBLK * BLKW], bf16, name=f"vb{i}")
            for i in range(VB_BUFS)
        ]
        for vb in vb_tiles:
            pads = vb[:].rearrange("p (b c) -> p b c", c=BLKW)[:, :, 0 : BLKW : BLKW - 1]
            nc.vector.memset(pads, 0.0)

        xg_tiles = {}

        def issue_load(gl):
            if gl >= ngroups:
                return
            xg = xin_pool.tile([H, GW], bf16)
            xg_tiles[gl] = xg
            # loads ride the ACT HWDGE ring (trigger never waits: the xin
            # buffer was freed long ago), so prefetch cannot get stuck
            # behind a store blocked on compute on the SP ring
            nc.scalar.dma_start(xg[:], x[gl])

        # Load schedule: burst BURST loads up front (fills the DMA's pre-store
        # ramp window), then ration one load every OTHER group so load
        # traffic lasts to the end of the run. Without this, loads exhaust
        # ~2/3 through and the per-group DMA work (stores only, ~2.9us)
        # drops below the compute period (~3.4us): the trace shows ~600ns
        # DMA idle per late group with both compute engines 100% busy.
        # Group gl's load issues at iteration 2*(gl-BURST), i.e. 34-gl
        # groups early -- still >= 3 groups of slack for the last group.
        for gl in range(BURST):
            issue_load(gl)

        for g in range(ngroups):
            if g % 2 == 0:
                issue_load(BURST + g // 2)
            xg = xg_tiles.pop(g)

            # pass 1 (vertical): V[p, (ph, i, j)] f32 in PSUM
            v = v_pool.tile([H, 2 * GW], f32)
            for ph in range(2):
                for half in range(2):
                    nc.tensor.matmul(
                        v[:, ph * GW + half * 512 : ph * GW + (half + 1) * 512],
                        A[:, ph * H : (ph + 1) * H],
                        xg[:, half * 512 : (half + 1) * 512],
                        start=True,
                        stop=True,
                    )
            v4 = v[:].rearrange("p (b j) -> p b j", b=NBLK)

            # u = 3V -> bf16 SBUF (ACT)
            u = u_pool.tile([H, 2 * GW], bf16)
            nc.scalar.mul(u[:], v[:], 3.0)

            # Vb = V -> bf16 into padded blocks, split ACT / DVE
            vb = vb_tiles[g % VB_BUFS]
            vb4 = vb[:].rearrange("p (b c) -> p b c", c=BLKW)
            ba = ACT_VB_BLOCKS
            if ba > 0:
                nc.scalar.copy(vb4[:, 0:ba, 1 : W + 1], v4[:, 0:ba, :])
            if ba < NBLK:
                nc.vector.tensor_copy(vb4[:, ba:NBLK, 1 : W + 1], v4[:, ba:NBLK, :])

            # pass 2 adds (DVE, bf16 2x): E = u + Vb[j-1], O = u + Vb[j+1];
            # store each plane as soon as its add completes (4 KB/partition)
            eo = eo_pool.tile([H, 2 * 2 * GW], bf16)
            eo4 = eo[:].rearrange("p (pl b j) -> p pl b j", pl=2, b=NBLK)
            u4 = u[:].rearrange("p (b j) -> p b j", b=NBLK)
            # single DVE op for both planes: pl dim is broadcast (stride 0)
            # on u and selects the +-1 shift (stride 2) on Vb
            u_ap = u4[:, :, :]
            ub = AP(u_ap.tensor, u_ap.offset, [list(u_ap.ap[0]), [0, 2]] + [list(d) for d in u_ap.ap[1:]])
            vbs = vb4[:, :, 0:W]
            vbb = AP(vbs.tensor, vbs.offset, [list(vbs.ap[0]), [2, 2]] + [list(d) for d in vbs.ap[1:]])
            nc.vector.tensor_add(eo4[:, 0:2], ub, vbb)
            # two stores per group (4 KB/partition each, matching the DMA
            # packet ceiling; a single 8 KB store measured ~3 us slower)
            out2 = out[g].rearrange("p (pl c) -> p pl c", pl=2)
            nc.sync.dma_start(out2[:, 0], eo[:, 0 : 2 * GW])
            nc.sync.dma_start(out2[:, 1], eo[:, 2 * GW : 4 * GW])


def build_bass(ngroups=NGROUPS, enable_asserts=False):
    import concourse.bacc as bacc
    import concourse.mybir as mybir
    import concourse.tile as tile

    bf16 = mybir.dt.bfloat16
    nc = bacc.Bacc(
        "TRN2",
        target_bir_lowering=False,
        debug=False,
        enable_asserts=enable_asserts,
        num_devices=N_CORES,
    )
    x = nc.dram_tensor("x", [ngroups, H, GROUP * W], bf16, kind="ExternalInput").ap()
    filt = nc.dram_tensor("filt", [H, 2 * H], bf16, kind="ExternalInput").ap()
    out = nc.dram_tensor(
        "out", [ngroups, H, 2 * NBLK * W], bf16, kind="ExternalOutput"
    ).ap()
    with tile.TileContext(nc) as tc:
        build_kernel_body(tc, x, filt, out, ngroups)
    nc.compile()
    return nc


_NC_CACHE = {}


def _prep_input_core(x_core):
    """[256, 128, 128] f32 -> [32, h, i, w] bf16 contiguous."""
    import ml_dtypes

    v = x_core.reshape(NGROUPS, GROUP, H, W).transpose(0, 2, 1, 3)
    return np.ascontiguousarray(v.astype(ml_dtypes.bfloat16))


def _unpack_output_core(raw, dst):
    """raw [32, 128, 2*NBLK*W] bf16 -> dst [256, 256, 256] f32."""
    a = raw.reshape(NGROUPS, H, 2, 2, GROUP, W)  # g p pl ph i j
    # dst[g*GROUP+i, 2p+ph, 2j+pl]
    np.copyto(
        dst.reshape(NGROUPS, GROUP, H, 2, W, 2),
        a.transpose(0, 4, 1, 3, 5, 2),
        casting="unsafe",
    )


def kernel(x: np.ndarray, _trace=False, _trace_cores=None) -> np.ndarray:
    from concurrent.futures import ThreadPoolExecutor

    import ml_dtypes

    from concourse.bass_utils import run_bass_kernel_spmd

    assert x.shape == (16, 128, H, W), x.shape
    xf = np.ascontiguousarray(x, dtype=np.float32).reshape(
        N_CORES, NIMG_PER_CORE, H, W
    )
    A = _filter_matrix().astype(ml_dtypes.bfloat16)

    with ThreadPoolExecutor(N_CORES) as ex:
        xcores = list(ex.map(_prep_input_core, [xf[k] for k in range(N_CORES)]))
    in_maps = [{"x": xcores[k], "filt": A} for k in range(N_CORES)]

    key = NGROUPS
    if key not in _NC_CACHE:
        _NC_CACHE[key] = build_bass()
    nc = _NC_CACHE[key]

    res = run_bass_kernel_spmd(
        nc,
        in_maps,
        core_ids=list(range(N_CORES)),
        trace=_trace,
        trace_cores=_trace_cores,
    )
    out = np.empty((N_CORES * NIMG_PER_CORE, 2 * H, 2 * W), np.float32)
    with ThreadPoolExecutor(N_CORES) as ex:
        list(
            ex.map(
                lambda k: _unpack_output_core(
                    res.results[k]["out"],
                    out[k * NIMG_PER_CORE : (k + 1) * NIMG_PER_CORE],
                ),
                range(N_CORES),
            )
        )
    if _trace:
        kernel._last_result = res
    return out.reshape(16, 128, 2 * H, 2 * W)


# revision 24
# speedup vs baseline: 1.0318x; 1.0196x over previous
"""Trainium2 Bass kernel for nn_Blur (upfirdn2d: up=2, pad=(2,1,2,1), 4-tap
separable filter [1,3,3,1] x [1,3,3,1] / 64).

Input  x [16, 128, 128, 128] f32  ->  Output [16, 128, 256, 256] f32.

Math (polyphase decomposition of the zero-insertion upsample + conv), per
axis with zero boundary:
  even outputs:  y[2i]   = (1*x[i-1] + 3*x[i]) / 8
  odd  outputs:  y[2i+1] = (3*x[i]   + 1*x[i+1]) / 8

The kernel is HBM-bound, so all device I/O is bf16 (gate is rel_err < 2e-2;
the full bf16 rounding chain measures ~3e-3):
  in  8 MB/core + out 32 MB/core = 40 MB/core @ ~358 GB/s  ->  ~112 us floor
(f32 baseline moved 80 MB/core -> 269 us).

Pipeline per group of GROUP=8 channel-images:
  pass 1 (vertical) on TensorE: V = A.T @ X, A [128, 256] bf16 banded
     polyphase matrix carrying the full 1/64 scale; V in PSUM f32 laid out
     [p, (ph, i, j)] (ph = output row parity, p = row pair index).
  pass 2 (horizontal) split for engine balance (ACT ~ DVE ~ 109 us each,
     both under the DMA floor):
   - ACT: u = 3V -> SBUF bf16 (one op), plus part of Vb = copy(V) into
     zero-padded 130-wide blocks (pads persist across groups: K rotating
     tiles memset once at startup).
   - DVE: rest of the Vb copy (1x, PSUM operand), then ONE fused add in
     bf16 2x mode (all operands 16-bit SBUF stride-1, 4B-aligned) over a
     plane dim that is broadcast (stride 0) on u and selects the +-1
     shift (stride 2 elements) on Vb:
        E[j] = u[j] + Vb[j-1]   (even output cols)
        O[j] = u[j] + Vb[j+1]   (odd  output cols)
     Zero pads make the j=0/127 boundaries fall out with no edge ops.
  store: two DMAs per group (E plane, O plane), 4 KB/partition bf16 each.

Host side: input is pre-permuted+cast to bf16 [32, h, i, w] per core so
loads are 2 KB/partition contiguous; output planes are gathered/interleaved
back to f32 NCHW with a threaded strided-cast pass.

Sharding: pure data parallel, 2 examples (256 channel-images) per core.
"""

import numpy as np

H = 128
W = 128
N_CORES = 8
NIMG_PER_CORE = 2 * 128  # 256 channel-images per core
GROUP = 8
NGROUPS = NIMG_PER_CORE // GROUP  # 32
NBLK = 2 * GROUP       # (ph, i) blocks per group
BLKW = W + 2           # padded Vb block width
LEAD = 12              # input loads issued this many groups ahead
VB_BUFS = 3            # persistent padded-Vb tiles (pads memset once)
ACT_VB_BLOCKS = 10     # Vb blocks copied by ACT; rest (NBLK-this) by DVE


def _filter_matrix() -> np.ndarray:
    """A[h, m] bf16-exact: m in 0..127 -> even output row 2m; m in 128..255
    -> odd row 2(m-128)+1. Carries the full 1/64 separable scale."""
    A = np.zeros((H, 2 * H), np.float32)
    for i in range(H):
        if i - 1 >= 0:
            A[i - 1, i] = 1.0 / 64
        A[i, i] = 3.0 / 64
        A[i, H + i] = 3.0 / 64
        if i + 1 < H:
            A[i + 1, H + i] = 1.0 / 64
    return A


def build_kernel_body(tc, x, filt, out, ngroups):
    """x [ngroups, 128, GROUP*W] bf16 (h-major, pre-permuted on host),
    filt [128, 256] bf16, out [ngroups, 128, 2*NBLK*W] bf16."""
    from contextlib import ExitStack

    import concourse.mybir as mybir
    from concourse.ap import AP

    bf16 = mybir.dt.bfloat16
    f32 = mybir.dt.float32
    nc = tc.nc
    GW = GROUP * W  # 1024

    with ExitStack() as ctx:
        const_pool = ctx.enter_context(tc.tile_pool(name="const", bufs=1))
        xin_pool = ctx.enter_context(tc.tile_pool(name="xin", bufs=LEAD + 2))
        v_pool = ctx.enter_context(tc.tile_pool(name="v", bufs=2, space="PSUM"))
        u_pool = ctx.enter_context(tc.tile_pool(name="u", bufs=4))
        vb_pool = ctx.enter_context(tc.tile_pool(name="vb", bufs=VB_BUFS))
        eo_pool = ctx.enter_context(tc.tile_pool(name="eo", bufs=4))

        A = const_pool.tile([H, 2 * H], bf16)
        nc.sync.dma_start(A[:], filt)

        # persistent padded-Vb tiles; zero the pad columns once
        vb_tiles = [
            vb_pool.tile([H, NBLK * BLKW], bf16, name=f"vb{i}")
            for i in range(VB_BUFS)
        ]
        for vb in vb_tiles:
            pads = vb[:].rearrange("p (b c) -> p b c", c=BLKW)[:, :, 0 : BLKW : BLKW - 1]
            nc.vector.memset(pads, 0.0)

        xg_tiles = {}

        def issue_load(gl):
            if gl >= ngroups:
                return
            xg = xin_pool.tile([H, GW], bf16)
            xg_tiles[gl] = xg
            # loads ride the ACT HWDGE ring (trigger never waits: the xin
            # buffer was freed long ago), so prefetch cannot get stuck
            # behind a store blocked on compute on the SP ring
            nc.scalar.dma_start(xg[:], x[gl])

        for gl in range(LEAD):
            issue_load(gl)

        for g in range(ngroups):
            issue_load(g + LEAD)
            xg = xg_tiles.pop(g)

            # pass 1 (vertical): V[p, (ph, i, j)] f32 in PSUM
            v = v_pool.tile([H, 2 * GW], f32)
            for ph in range(2):
                for half in range(2):
                    nc.tensor.matmul(
                        v[:, ph * GW + half * 512 : ph * GW + (half + 1) * 512],
                        A[:, ph * H : (ph + 1) * H],
                        xg[:, half * 512 : (half + 1) * 512],
                        start=True,
                        stop=True,
                    )
            v4 = v[:].rearrange("p (b j) -> p b j", b=NBLK)

            # u = 3V -> bf16 SBUF (ACT)
            u = u_pool.tile([H, 2 * GW], bf16)
            nc.scalar.mul(u[:], v[:], 3.0)

            # Vb = V -> bf16 into padded blocks, split ACT / DVE
            vb = vb_tiles[g % VB_BUFS]
            vb4 = vb[:].rearrange("p (b c) -> p b c", c=BLKW)
            ba = ACT_VB_BLOCKS
            if ba > 0:
                nc.scalar.copy(vb4[:, 0:ba, 1 : W + 1], v4[:, 0:ba, :])
            if ba < NBLK:
                nc.vector.tensor_copy(vb4[:, ba:NBLK, 1 : W + 1], v4[:, ba:NBLK, :])

            # pass 2 adds (DVE, bf16 2x): E = u + Vb[j-1], O = u + Vb[j+1];
            # store each plane as soon as its add completes (4 KB/partition)
            eo = eo_pool.tile([H, 2 * 2 * GW], bf16)
            eo4 = eo[:].rearrange("p (pl b j) -> p pl b j", pl=2, b=NBLK)
            u4 = u[:].rearrange("p (b j) -> p b j", b=NBLK)
            # single DVE op for both planes: pl dim is broadcast (stride 0)
            # on u and selects the +-1 shift (stride 2) on Vb
            u_ap = u4[:, :, :]
            ub = AP(u_ap.tensor, u_ap.offset, [list(u_ap.ap[0]), [0, 2]] + [list(d) for d in u_ap.ap[1:]])
            vbs = vb4[:, :, 0:W]
            vbb = AP(vbs.tensor, vbs.offset, [list(vbs.ap[0]), [2, 2]] + [list(d) for d in vbs.ap[1:]])
            nc.vector.tensor_add(eo4[:, 0:2], ub, vbb)
            # two stores per group (4 KB/partition each, matching the DMA
            # packet ceiling; a single 8 KB store measured ~3 us slower)
            out2 = out[g].rearrange("p (pl c) -> p pl c", pl=2)
            nc.sync.dma_start(out2[:, 0], eo[:, 0 : 2 * GW])
            nc.sync.dma_start(out2[:, 1], eo[:, 2 * GW : 4 * GW])


def build_bass(ngroups=NGROUPS, enable_asserts=False):
    import concourse.bacc as bacc
    import concourse.mybir as mybir
    import concourse.tile as tile

    bf16 = mybir.dt.bfloat16
    nc = bacc.Bacc(
        "TRN2",
        target_bir_lowering=False,
        debug=False,
        enable_asserts=enable_asserts,
        num_devices=N_CORES,
    )
    x = nc.dram_tensor("x", [ngroups, H, GROUP * W], bf16, kind="ExternalInput").ap()
    filt = nc.dram_tensor("filt", [H, 2 * H], bf16, kind="ExternalInput").ap()
    out = nc.dram_tensor(
        "out", [ngroups, H, 2 * NBLK * W], bf16, kind="ExternalOutput"
    ).ap()
    with tile.TileContext(nc) as tc:
        build_kernel_body(tc, x, filt, out, ngroups)
    nc.compile()
    return nc


_NC_CACHE = {}


def _prep_input_core(x_core):
    """[256, 128, 128] f32 -> [32, h, i, w] bf16 contiguous."""
    import ml_dtypes

    v = x_core.reshape(NGROUPS, GROUP, H, W).transpose(0, 2, 1, 3)
    return np.ascontiguousarray(v.astype(ml_dtypes.bfloat16))


def _unpack_output_core(raw, dst):
    """raw [32, 128, 2*NBLK*W] bf16 -> dst [256, 256, 256] f32."""
    a = raw.reshape(NGROUPS, H, 2, 2, GROUP, W)  # g p pl ph i j
    # dst[g*GROUP+i, 2p+ph, 2j+pl]
    np.copyto(
        dst.reshape(NGROUPS, GROUP, H, 2, W, 2),
        a.transpose(0, 4, 1, 3, 5, 2),
        casting="unsafe",
    )


def kernel(x: np.ndarray, _trace=False, _trace_cores=None) -> np.ndarray:
    from concurrent.futures import ThreadPoolExecutor

    import ml_dtypes

    from concourse.bass_utils import run_bass_kernel_spmd

    assert x.shape == (16, 128, H, W), x.shape
    xf = np.ascontiguousarray(x, dtype=np.float32).reshape(
        N_CORES, NIMG_PER_CORE, H, W
    )
    A = _filter_matrix().astype(ml_dtypes.bfloat16)

    with ThreadPoolExecutor(N_CORES) as ex:
        xcores = list(ex.map(_prep_input_core, [xf[k] for k in range(N_CORES)]))
    in_maps = [{"x": xcores[k], "filt": A} for k in range(N_CORES)]

    key = NGROUPS
    if key not in _NC_CACHE:
        _NC_CACHE[key] = build_bass()
    nc = _NC_CACHE[key]

    res = run_bass_kernel_spmd(
        nc,
        in_maps,
        core_ids=list(range(N_CORES)),
        trace=_trace,
        trace_cores=_trace_cores,
    )
    out = np.empty((N_CORES * NIMG_PER_CORE, 2 * H, 2 * W), np.float32)
    with ThreadPoolExecutor(N_CORES) as ex:
        list(
            ex.map(
                lambda k: _unpack_output_core(
                    res.results[k]["out"],
                    out[k * NIMG_PER_CORE : (k + 1) * NIMG_PER_CORE],
                ),
                range(N_CORES),
            )
        )
    if _trace:
        kernel._last_result = res
    return out.reshape(16, 128, 2 * H, 2 * W)
